# revision 9
# baseline (speedup 1.0000x reference)
"""Conv2d 3x3 (stride 1, pad 1) forward on 8 Trainium2 NeuronCores.

Problem: x (32,32,128,128) f32, kernel (64,32,3,3), bias (64)
         -> out (32,64,128,128).  Data-parallel: 4 images per core.

Per-core design (fp16 compute, fp32 PSUM accumulate):
  - x and kernel are cast to fp16 on the HOST and the one-pixel zero
    halo is materialized there too, so the device loads x as
    [Cin, 130, 130] fp16 with one fully-contiguous descriptor per
    partition.  Output is written fp16 and upcast on the host.
    (PSUM accumulation stays fp32; rel-err ~1e-3 vs the 2e-2 gate.)
  - Full 128x128 PE utilization: each of the 4 images is pinned to a
    32-partition row group (K = Cin = 32), and each image is split
    into TOP (rows 0..63) on PE columns 0..63 and BOTTOM (rows
    64..127) on PE columns 64..127.  8 concurrent 32x64 matmul
    streams via tile_position (32r, 64j).
  - A 3x3 conv is 9 shifted matmuls accumulated in PSUM.  Per round
    every stream produces 4 output rows (N = 512 = 1 PSUM bank), so
    each round yields 8 rows per image; 16 rounds total.
  - Drain: one [128, 512] op per image per round: PSUM bank holds
    (top half, co) on partitions 0..63 and (bottom half, co) on
    64..127.  ScalarE (activation+bias) drains images 0,1; VectorE
    (tensor_scalar_add) drains images 2,3.  Output staged fp16.
  - Stores are batched 2 rounds per DMA (1 MiB, 2 KiB contiguous
    runs); the final pair is stored as two 512 KiB transfers to
    shorten the tail.
"""
import sys
sys.path.insert(0, '/opt/trn_rl_repo')
import numpy as np

B, Cin, H, W = 32, 32, 128, 128
Cout, KH, KW = 64, 3, 3
NCORES = 8
BPC = B // NCORES          # images per core
Hp, Wp = H + 2, W + 2
HH = H // 2                # rows per image half
NTAP = KH * KW
ROWS_PER_ROUND = 4
NROUND = HH // ROWS_PER_ROUND   # 16

_cache = {}


def _build_program():
    from concourse import bacc
    import concourse.mybir as mybir
    from concourse.tile import TileContext

    f32 = mybir.dt.float32
    f16 = mybir.dt.float16
    Act = mybir.ActivationFunctionType

    nc = bacc.Bacc("TRN2", target_bir_lowering=False, debug=False,
                   num_devices=NCORES)
    x_ext = nc.declare_dram_parameter("x", [BPC * Cin, Hp, Wp], f16,
                                      isOutput=False)
    w_ext = nc.declare_dram_parameter("w", [128, NTAP, Cout], f16,
                                      isOutput=False)
    b_ext = nc.declare_dram_parameter("b", [128, 1], f32, isOutput=False)
    # native store layout: partition p = (half, co); host untangles it
    out_ext = nc.declare_dram_parameter("out", [128, BPC, HH, W], f16,
                                        isOutput=True)

    with TileContext(nc) as tc:
        with tc.tile_pool(name="xp", bufs=1) as xpool, \
             tc.tile_pool(name="const", bufs=1) as cpool, \
             tc.tile_pool(name="stage", bufs=3) as opool, \
             tc.tile_pool(name="psum", bufs=8, space="PSUM") as ppool:

            xp = xpool.tile([128, Hp, Wp], f16)
            wt = cpool.tile([128, NTAP, Cout], f16)
            bt = cpool.tile([128, 1], f32)

            nc.sync.dma_start(out=wt[:], in_=w_ext[:])
            nc.sync.dma_start(out=bt[:], in_=b_ext[:])

            # fully-contiguous loads, alternating top-band / bottom-band
            # chunks so both image halves fill in lockstep; first chunks
            # are small so round 0 can start early (round k needs top
            # rows <= 4k+5 and bottom rows <= 64+4k+5)
            tops = [(0, 10), (10, 26), (26, 46), (46, 66)]
            bots = [(64, 74), (74, 90), (90, 110), (110, 130)]
            for (t0, t1), (b0, b1) in zip(tops, bots):
                nc.sync.dma_start(out=xp[:, t0:t1, :], in_=x_ext[:, t0:t1, :])
                nc.sync.dma_start(out=xp[:, b0:b1, :], in_=x_ext[:, b0:b1, :])

            # partition p = (half, co); free = (img, rows-within-half, w)
            out_v = out_ext.rearrange("p img hh w -> p img (hh w)")

            R = ROWS_PER_ROUND
            ost = None
            for k in range(NROUND):
                h0 = k * R
                ps = [ppool.tile([128, R, W], f32, tag="ps",
                                 name=f"ps{k}_{r}")
                      for r in range(BPC)]
                for t in range(NTAP):
                    kh, kw = divmod(t, 3)
                    for r in range(BPC):
                        for j in range(2):
                            hb = HH * j + h0 + kh
                            nc.tensor.matmul(
                                ps[r][64 * j:64 * j + 64, :, :],
                                wt[32 * r:32 * r + 32, t, :],
                                xp[32 * r:32 * r + 32, hb:hb + R,
                                   kw:kw + W],
                                start=(t == 0), stop=(t == NTAP - 1),
                                tile_position=(32 * r, 64 * j))

                sub = k % 2
                if sub == 0:
                    ost = opool.tile([128, BPC, 2 * R, W], f16, tag="ost")
                sl = slice(sub * R, (sub + 1) * R)
                # one [128, 512] drain per image: partitions already
                # laid out as (half, co)
                nc.scalar.activation(ost[:, 0, sl, :], ps[0][:, :, :],
                                     Act.Identity, bias=bt[:, :])
                nc.scalar.activation(ost[:, 1, sl, :], ps[1][:, :, :],
                                     Act.Identity, bias=bt[:, :])
                nc.vector.tensor_scalar_add(ost[:, 2, sl, :],
                                            ps[2][:, :, :], bt[:, :])
                nc.vector.tensor_scalar_add(ost[:, 3, sl, :],
                                            ps[3][:, :, :], bt[:, :])

                if k == NROUND - 1:
                    # final round: store per engine-pair as soon as each
                    # drain lands, to shorten the tail
                    nc.sync.dma_start(
                        out=out_v[:, 0:2, h0 * W:(h0 + R) * W],
                        in_=ost[:, 0:2, sl, :])
                    nc.sync.dma_start(
                        out=out_v[:, 2:4, h0 * W:(h0 + R) * W],
                        in_=ost[:, 2:4, sl, :])
                elif k == NROUND - 2:
                    nc.sync.dma_start(
                        out=out_v[:, :, h0 * W:(h0 + R) * W],
                        in_=ost[:, :, sl, :])
                elif sub == 1:
                    # one 1-MiB store for 2 rounds x 4 images x 8 rows
                    nc.sync.dma_start(
                        out=out_v[:, :, (h0 - R) * W:(h0 + R) * W],
                        in_=ost[:, :, :, :])

    nc.compile()
    return nc


def _get_program():
    if "nc" not in _cache:
        _cache["nc"] = _build_program()
    return _cache["nc"]


def _prep_inputs(x, kernel, bias):
    # weights: (Cout, Cin, KH, KW) -> [ci, tap, co], replicated on the
    # 4 PE row groups
    w = np.transpose(kernel.reshape(Cout, Cin, NTAP), (1, 2, 0))
    w = np.ascontiguousarray(np.tile(w, (4, 1, 1))).astype(np.float16)
    b = np.ascontiguousarray(
        np.tile(bias.astype(np.float32), 2)[:, None])
    xpad = np.zeros((B * Cin, Hp, Wp), dtype=np.float16)
    xpad[:, 1:1 + H, 1:1 + W] = x.reshape(B * Cin, H, W).astype(np.float16)
    in_maps = []
    for c in range(NCORES):
        xs = xpad[c * BPC * Cin:(c + 1) * BPC * Cin]
        in_maps.append({"x": xs, "w": w, "b": b})
    return in_maps


def _run(inputs, trace=False):
    from concourse.bass_utils import run_bass_kernel_spmd
    nc = _get_program()
    in_maps = _prep_inputs(inputs["x"], inputs["kernel"], inputs["bias"])
    res = run_bass_kernel_spmd(nc, in_maps, list(range(NCORES)), trace=trace)
    parts = []
    for c in range(NCORES):
        od = res.results[c]["out"].astype(np.float32)
        od = od.reshape(2, Cout, BPC, HH, W)        # [half, co, img, hh, w]
        parts.append(od.transpose(2, 1, 0, 3, 4).reshape(BPC, Cout, H, W))
    return np.concatenate(parts, axis=0), res


def kernel(**inputs):
    out, _ = _run(inputs, trace=False)
    return out
